# revision 7
# baseline (speedup 1.0000x reference)
"""GAT (nn_GAT_29523605193094) Trainium2 kernel.

The reference keeps the source bug ``src, dst = edges[0], edges[0]``, so the
adjacency matrix is purely diagonal: adj[i, i] = (i appears in edges[0]).
After the -inf masking, row i of the [N, N, H] score tensor has exactly one
finite entry (j = i) when node i is covered, so softmax over axis=1 yields
exactly 1.0 at (i, i) and 0.0 elsewhere, and the output row is exactly
h[i] = (X @ W)[i].  Rows for uncovered nodes are all -inf -> softmax is NaN
-> the output row is NaN.  Both cases are reproduced here:

    out = X @ W            (on 8 NeuronCores, row-sharded)
    out[~covered] = NaN    (host-side mask from edges[0])

The device work is a row-sharded [4096, 512] @ [512, 256] matmul, run in
bf16 (inputs cast on host; fp32 PSUM accumulation).  The fp32 harness
tolerance is 2e-2 relative to absmax(expected); bf16 lands at ~4.2e-3.

Per-core schedule notes (calibrated against NTFF profiles):
- Inputs are packed on host into partition-major [128, k, cols] layouts so
  each dma_start is 128 descriptors of 2-4KB contiguous lines (descriptor
  generation and SDMA line rate are the input-latency limiters).
- W rides the sync HWDGE ring (first doorbell after the preamble barrier);
  the X^T halves ride the scalar ring; outputs split across both rings.
- The PE HAM clock gate holds the array at 1.2 GHz until it has been busy
  ~3.4us.  Eight dummy matmuls on a zeroed scratch tile run during the
  input-DMA window so the real matmuls execute at 2.4 GHz.
- W k-chunks [128, 128] are PE-stationary; X^T chunks [128, 512] stream as
  the moving operand, accumulating into 2 PSUM banks (c = output column
  block).  The output leaves the device transposed ([OUT, RB] = h^T).
"""

import numpy as np
import ml_dtypes

N = 4096
IN = 512
OUT = 256
NCORES = 8
RB = N // NCORES  # 512 rows per core
P = 128
KT = IN // P      # 4 contraction chunks
CT = OUT // P     # 2 output column blocks
WARM = 0          # dummy matmuls to lift the PE HAM clock gate

_state = {}

# test.py reads this after a traced call for the HW exec time.
LAST_RESULTS = None


def _build():
    import concourse.mybir as mybir
    import concourse.tile as tile
    from concourse import bacc
    from concourse.bass import ts

    nc = bacc.Bacc(
        "TRN2",
        target_bir_lowering=False,
        debug=False,
        num_devices=NCORES,
    )
    f32 = mybir.dt.float32
    bf16 = mybir.dt.bfloat16
    # Partition-major packed inputs (see kernel()): 2-4KB lines per partition.
    xt = nc.dram_tensor("xt", [P, KT, RB], bf16, kind="ExternalInput")
    w = nc.dram_tensor("w", [P, KT, OUT], bf16, kind="ExternalInput")
    out = nc.dram_tensor("out", [OUT, RB], bf16, kind="ExternalOutput")  # h^T

    with tile.TileContext(nc) as tc:
        with (
            tc.tile_pool(name="ins", bufs=1) as in_pool,
            tc.tile_pool(name="outs", bufs=2) as out_pool,
            tc.tile_pool(name="ps", bufs=3, space="PSUM") as psum_pool,
        ):
            xt_t = in_pool.tile([P, KT, RB], bf16)
            w_t = in_pool.tile([P, KT, OUT], bf16)
            scratch = in_pool.tile([P, RB], bf16)

            if WARM:
                # PE warm-up: memset scratch on the (otherwise idle) gpsimd
                # engine, then stream dummy matmuls so the HAM un-throttles
                # the PE clock before the real matmuls arrive.
                nc.gpsimd.memset(scratch[:], 0.0)
                ps_warm = psum_pool.tile([P, RB], f32, name="ps_warm", tag="psw")
                for _ in range(WARM):
                    nc.tensor.matmul(
                        ps_warm[:], scratch[:, 0:P], scratch[:], start=True, stop=True
                    )

            # All inputs ride the sync ring FIFO in consumption order
            # (cross-ring transfers start ~1us staggered, single-ring flows
            # continuously): W whole, then X^T k0, k1, k2-3 so the first
            # matmul starts as soon as w+k0 land while the rest stream.
            nc.sync.dma_start(w_t[:], w[:, :, :])
            nc.sync.dma_start(xt_t[:, 0:1, :], xt[:, 0:1, :])
            nc.sync.dma_start(xt_t[:, 1:2, :], xt[:, 1:2, :])
            nc.sync.dma_start(xt_t[:, 2:4, :], xt[:, 2:4, :])

            for c in range(CT):
                ps = psum_pool.tile([P, RB], f32, name=f"ps{c}", tag="ps")
                for k in range(KT):
                    nc.tensor.matmul(
                        ps[:],
                        w_t[:, k, ts(c, P)],
                        xt_t[:, k, :],
                        start=(k == 0),
                        stop=(k == KT - 1),
                    )
                ob = out_pool.tile([P, RB], bf16)
                if c == 0:
                    # c0 overlaps c1's matmuls: full copy, scalar ring.
                    nc.vector.tensor_copy(ob[:], ps[:])
                    nc.scalar.dma_start(out[ts(c, P), :], ob[:])
                else:
                    # Tail: cast halves so the first half's DMA (sync ring)
                    # issues while the second half casts, then the second
                    # half rides the scalar ring; the write receipts overlap.
                    HB = RB // 2
                    nc.vector.tensor_copy(ob[:, 0:HB], ps[:, 0:HB])
                    nc.sync.dma_start(out[ts(c, P), 0:HB], ob[:, 0:HB])
                    nc.vector.tensor_copy(ob[:, HB:RB], ps[:, HB:RB])
                    nc.scalar.dma_start(out[ts(c, P), HB:RB], ob[:, HB:RB])

    nc.compile()
    return nc


def kernel(X, edges, W, A):
    global LAST_RESULTS
    from concourse.bass_utils import run_bass_kernel_spmd

    X = np.asarray(X, dtype=np.float32)
    W = np.asarray(W, dtype=np.float32)
    edges = np.asarray(edges)

    if "nc" not in _state:
        _state["nc"] = _build()
    nc = _state["nc"]

    # Pack to partition-major [128, k, cols]: row p holds chunk-k data for
    # SBUF partition p, so each DMA line is one long contiguous run.
    XT = np.ascontiguousarray(X.T).astype(ml_dtypes.bfloat16)  # [IN, N]
    Wp = np.ascontiguousarray(
        W.astype(ml_dtypes.bfloat16).reshape(KT, P, OUT).transpose(1, 0, 2)
    )  # [128, KT, OUT]
    in_maps = []
    for c in range(NCORES):
        shard = XT[:, c * RB : (c + 1) * RB]  # [IN, RB]
        xp = np.ascontiguousarray(
            shard.reshape(KT, P, RB).transpose(1, 0, 2)
        )  # [128, KT, RB]
        in_maps.append({"xt": xp, "w": Wp})
    # The device occasionally reports a transient NRT_EXEC_UNIT_UNRECOVERABLE
    # on an otherwise-good kernel; retry before giving up.
    last_exc = None
    for _attempt in range(3):
        try:
            res = run_bass_kernel_spmd(nc, in_maps, core_ids=list(range(NCORES)))
            break
        except Exception as exc:  # noqa: BLE001
            last_exc = exc
            import time

            time.sleep(2.0)
    else:
        raise last_exc
    LAST_RESULTS = res
    # Per-core output is h_shard^T [OUT, RB]; stitch columns then transpose.
    out_t = np.concatenate(
        [np.asarray(res.results[c]["out"]) for c in range(NCORES)], axis=1
    )  # [OUT, N]
    out = out_t.T.astype(np.float32)

    # Reference semantics: nodes absent from edges[0] have an all -inf score
    # row; softmax of that is NaN, which propagates to the output row.
    covered = np.zeros(N, dtype=bool)
    covered[edges[0]] = True
    if not covered.all():
        out[~covered] = np.nan
    return out


# revision 8
# speedup vs baseline: 1.0899x; 1.0899x over previous
"""GAT (nn_GAT_29523605193094) Trainium2 kernel.

The reference keeps the source bug ``src, dst = edges[0], edges[0]``, so the
adjacency matrix is purely diagonal: adj[i, i] = (i appears in edges[0]).
After the -inf masking, row i of the [N, N, H] score tensor has exactly one
finite entry (j = i) when node i is covered, so softmax over axis=1 yields
exactly 1.0 at (i, i) and 0.0 elsewhere, and the output row is exactly
h[i] = (X @ W)[i].  Rows for uncovered nodes are all -inf -> softmax is NaN
-> the output row is NaN.  Both cases are reproduced here:

    out = X @ W            (on 8 NeuronCores, row-sharded)
    out[~covered] = NaN    (host-side mask from edges[0])

The device work is a row-sharded [4096, 512] @ [512, 256] matmul, run in
bf16 (inputs cast on host; fp32 PSUM accumulation).  The fp32 harness
tolerance is 2e-2 relative to absmax(expected); bf16 lands at ~4.2e-3.

Per-core schedule notes (calibrated against NTFF profiles):
- Inputs are packed on host into partition-major [128, k, cols] layouts so
  each dma_start is 128 descriptors of 2-4KB contiguous lines (descriptor
  generation and SDMA line rate are the input-latency limiters).
- W rides the sync HWDGE ring (first doorbell after the preamble barrier);
  the X^T halves ride the scalar ring; outputs split across both rings.
- The PE HAM clock gate holds the array at 1.2 GHz until it has been busy
  ~3.4us.  Eight dummy matmuls on a zeroed scratch tile run during the
  input-DMA window so the real matmuls execute at 2.4 GHz.
- W k-chunks [128, 128] are PE-stationary; X^T chunks [128, 512] stream as
  the moving operand, accumulating into 2 PSUM banks (c = output column
  block).  The output leaves the device transposed ([OUT, RB] = h^T).
"""

import numpy as np
import ml_dtypes

N = 4096
IN = 512
OUT = 256
NCORES = 8
RB = N // NCORES  # 512 rows per core
P = 128
KT = IN // P      # 4 contraction chunks
CT = OUT // P     # 2 output column blocks
WARM = 0          # dummy matmuls to lift the PE HAM clock gate

_state = {}

# test.py reads this after a traced call for the HW exec time.
LAST_RESULTS = None


def _build():
    import concourse.mybir as mybir
    import concourse.tile as tile
    from concourse import bacc
    from concourse.bass import ts

    nc = bacc.Bacc(
        "TRN2",
        target_bir_lowering=False,
        debug=False,
        num_devices=NCORES,
    )
    f32 = mybir.dt.float32
    bf16 = mybir.dt.bfloat16
    # Partition-major packed inputs (see kernel()): 2-4KB lines per partition.
    xt = nc.dram_tensor("xt", [P, KT, RB], bf16, kind="ExternalInput")
    w = nc.dram_tensor("w", [P, KT, OUT], bf16, kind="ExternalInput")
    out = nc.dram_tensor("out", [OUT, RB], bf16, kind="ExternalOutput")  # h^T

    with tile.TileContext(nc) as tc:
        with (
            tc.tile_pool(name="ins", bufs=1) as in_pool,
            tc.tile_pool(name="outs", bufs=2) as out_pool,
            tc.tile_pool(name="ps", bufs=3, space="PSUM") as psum_pool,
        ):
            xt_t = in_pool.tile([P, KT, RB], bf16)
            w_t = in_pool.tile([P, KT, OUT], bf16)
            scratch = in_pool.tile([P, RB], bf16)

            if WARM:
                # PE warm-up: memset scratch on the (otherwise idle) gpsimd
                # engine, then stream dummy matmuls so the HAM un-throttles
                # the PE clock before the real matmuls arrive.
                nc.gpsimd.memset(scratch[:], 0.0)
                ps_warm = psum_pool.tile([P, RB], f32, name="ps_warm", tag="psw")
                for _ in range(WARM):
                    nc.tensor.matmul(
                        ps_warm[:], scratch[:, 0:P], scratch[:], start=True, stop=True
                    )

            # Inputs balanced across both HWDGE rings (each ring tops out
            # ~150GB/s; the second ring starts ~1us after the first), in
            # consumption order: sync ring leads with W then k0, k1; the
            # scalar ring carries k2-3 which are needed last.
            nc.sync.dma_start(w_t[:], w[:, :, :])
            nc.sync.dma_start(xt_t[:, 0:1, :], xt[:, 0:1, :])
            nc.sync.dma_start(xt_t[:, 1:2, :], xt[:, 1:2, :])
            nc.scalar.dma_start(xt_t[:, 2:4, :], xt[:, 2:4, :])

            for c in range(CT):
                ps = psum_pool.tile([P, RB], f32, name=f"ps{c}", tag="ps")
                for k in range(KT):
                    nc.tensor.matmul(
                        ps[:],
                        w_t[:, k, ts(c, P)],
                        xt_t[:, k, :],
                        start=(k == 0),
                        stop=(k == KT - 1),
                    )
                ob = out_pool.tile([P, RB], bf16)
                if c == 0:
                    # c0 overlaps c1's matmuls: full copy, scalar ring.
                    nc.vector.tensor_copy(ob[:], ps[:])
                    nc.scalar.dma_start(out[ts(c, P), :], ob[:])
                else:
                    # Tail: cast halves so the first half's DMA (sync ring)
                    # issues while the second half casts, then the second
                    # half rides the scalar ring; the write receipts overlap.
                    HB = RB // 2
                    nc.vector.tensor_copy(ob[:, 0:HB], ps[:, 0:HB])
                    nc.sync.dma_start(out[ts(c, P), 0:HB], ob[:, 0:HB])
                    nc.vector.tensor_copy(ob[:, HB:RB], ps[:, HB:RB])
                    nc.scalar.dma_start(out[ts(c, P), HB:RB], ob[:, HB:RB])

    nc.compile()
    return nc


def kernel(X, edges, W, A):
    global LAST_RESULTS
    from concourse.bass_utils import run_bass_kernel_spmd

    X = np.asarray(X, dtype=np.float32)
    W = np.asarray(W, dtype=np.float32)
    edges = np.asarray(edges)

    if "nc" not in _state:
        _state["nc"] = _build()
    nc = _state["nc"]

    # Pack to partition-major [128, k, cols]: row p holds chunk-k data for
    # SBUF partition p, so each DMA line is one long contiguous run.
    XT = np.ascontiguousarray(X.T).astype(ml_dtypes.bfloat16)  # [IN, N]
    Wp = np.ascontiguousarray(
        W.astype(ml_dtypes.bfloat16).reshape(KT, P, OUT).transpose(1, 0, 2)
    )  # [128, KT, OUT]
    in_maps = []
    for c in range(NCORES):
        shard = XT[:, c * RB : (c + 1) * RB]  # [IN, RB]
        xp = np.ascontiguousarray(
            shard.reshape(KT, P, RB).transpose(1, 0, 2)
        )  # [128, KT, RB]
        in_maps.append({"xt": xp, "w": Wp})
    # The device occasionally reports a transient NRT_EXEC_UNIT_UNRECOVERABLE
    # on an otherwise-good kernel; retry before giving up.
    last_exc = None
    for _attempt in range(3):
        try:
            res = run_bass_kernel_spmd(nc, in_maps, core_ids=list(range(NCORES)))
            break
        except Exception as exc:  # noqa: BLE001
            last_exc = exc
            import time

            time.sleep(2.0)
    else:
        raise last_exc
    LAST_RESULTS = res
    # Per-core output is h_shard^T [OUT, RB]; stitch columns then transpose.
    out_t = np.concatenate(
        [np.asarray(res.results[c]["out"]) for c in range(NCORES)], axis=1
    )  # [OUT, N]
    out = out_t.T.astype(np.float32)

    # Reference semantics: nodes absent from edges[0] have an all -inf score
    # row; softmax of that is NaN, which propagates to the output row.
    covered = np.zeros(N, dtype=bool)
    covered[edges[0]] = True
    if not covered.all():
        out[~covered] = np.nan
    return out
